# revision 1
# baseline (speedup 1.0000x reference)
import numpy as np

# nn_Attention3D: hardcoded problem shapes
B, DIM, N, H, W = 8, 64, 32, 32, 32
HEADS = 8
CH = DIM // HEADS          # 8 channels per head
S = N * H * W              # 32768 spatial
EPS = 1e-12


def _forward_np(x, w_qkv, w_dw, w_proj, temperature):
    """Vectorized numpy reference-equivalent (fallback path)."""
    wq = w_qkv[:, :, 0, 0, 0]            # (192, 64)
    wd = w_dw[:, 0]                      # (192, 3, 3, 3)
    wp = w_proj[:, :, 0, 0, 0]           # (64, 64)

    xf = x.reshape(B, DIM, S)
    qkv = np.einsum('oi,bis->bos', wq, xf).reshape(B, 3 * DIM, N, H, W)

    qp = np.pad(qkv, ((0, 0), (0, 0), (1, 1), (1, 1), (1, 1)))
    acc = np.zeros_like(qkv)
    for dz in range(3):
        for dy in range(3):
            for dx in range(3):
                acc += wd[:, dz, dy, dx][None, :, None, None, None] * \
                    qp[:, :, dz:dz + N, dy:dy + H, dx:dx + W]

    q, k, v = np.split(acc.reshape(B, 3 * DIM, S), 3, axis=1)
    rs = lambda t: t.reshape(B, HEADS, CH, S)
    q, k, v = rs(q), rs(k), rs(v)
    l2 = lambda t: t / np.maximum(
        np.sqrt((t * t).sum(-1, keepdims=True)), EPS)
    q, k = l2(q), l2(k)

    temp = temperature.reshape(-1)[:B].astype(np.float32)  # per-batch scalar
    logits = np.einsum('bhcs,bhds->bhcd', q, k) * temp[:, None, None, None]
    logits -= logits.max(-1, keepdims=True)
    e = np.exp(logits)
    attn = e / e.sum(-1, keepdims=True)

    out = np.einsum('bhcd,bhds->bhcs', attn, v).reshape(B, DIM, S)
    y = np.einsum('oi,bis->bos', wp, out)
    return y.reshape(B, DIM, N, H, W).astype(np.float32)


def _forward_jax(x, w_qkv, w_dw, w_proj, temperature):
    """Data-parallel over batch: one batch element per NeuronCore (8 cores)."""
    import jax
    import jax.numpy as jnp

    devs = jax.devices()[:8]
    if len(devs) < 8:
        raise RuntimeError('need 8 devices')

    wq = jnp.asarray(w_qkv[:, :, 0, 0, 0])
    wd = jnp.asarray(w_dw[:, 0])
    wp = jnp.asarray(w_proj[:, :, 0, 0, 0])
    temp = jnp.asarray(temperature.reshape(-1)[:B])

    def one(xb, tb, wq, wd, wp):
        qkv = jnp.einsum('oi,is->os', wq, xb.reshape(DIM, S))
        qkv = qkv.reshape(3 * DIM, N, H, W)
        qp = jnp.pad(qkv, ((0, 0), (1, 1), (1, 1), (1, 1)))
        acc = jnp.zeros((3 * DIM, N, H, W), jnp.float32)
        for dz in range(3):
            for dy in range(3):
                for dx in range(3):
                    acc = acc + wd[:, dz, dy, dx][:, None, None, None] * \
                        qp[:, dz:dz + N, dy:dy + H, dx:dx + W]
        q, k, v = jnp.split(acc.reshape(3 * DIM, S), 3, axis=0)
        rs = lambda t: t.reshape(HEADS, CH, S)
        q, k, v = rs(q), rs(k), rs(v)
        l2 = lambda t: t / jnp.maximum(
            jnp.sqrt((t * t).sum(-1, keepdims=True)), EPS)
        q, k = l2(q), l2(k)
        attn = jax.nn.softmax(jnp.einsum('hcs,hds->hcd', q, k) * tb, axis=-1)
        out = jnp.einsum('hcd,hds->hcs', attn, v).reshape(DIM, S)
        return jnp.einsum('oi,is->os', wp, out).reshape(DIM, N, H, W)

    f = jax.pmap(one, in_axes=(0, 0, None, None, None), devices=devs)
    y = f(jnp.asarray(x), temp, wq, wd, wp)
    return np.asarray(y).astype(np.float32)


def kernel(x, w_qkv, w_dw, w_proj, temperature):
    x = np.asarray(x, np.float32)
    w_qkv = np.asarray(w_qkv, np.float32)
    w_dw = np.asarray(w_dw, np.float32)
    w_proj = np.asarray(w_proj, np.float32)
    temperature = np.asarray(temperature, np.float32)
    try:
        return _forward_jax(x, w_qkv, w_dw, w_proj, temperature)
    except Exception:
        return _forward_np(x, w_qkv, w_dw, w_proj, temperature)

